# revision 40
# baseline (speedup 1.0000x reference)
"""Trainium2 Bass kernel for the CandidateFinder sparse-attention problem.

Computes, for each (batch, query) row, the first K_MAX=64 key indices whose
32-bit sign pattern exactly matches the query's in either of two dim groups
(dims 0:32, 32:64), padded with -1.

Approach (per core; 8 cores = 4 batches x 2 query halves):
  - signs s = 2*(x>0)-1 in bf16 (exact); per group S_g[q,j] = sum_d s_q s_k
    is an integer in [-32,32]; match <=> S_g == 32. (TensorE, K=34.)
  - two extra contraction rows add ramp(j) = (2048-j)*2^-13 (sum of two
    bf16-exact products), so S'_g = S_g + ramp is exact in fp32 PSUM and
    strictly decreasing in j for fixed S.
  - ScalarE evacuates group 2 as relu(S'_2 - 32) -> fp16 (matched positions
    give exactly (2048-j)*2^-13, fp16-exact and descending in j; rest 0);
    a fused DVE op evacuates group 1 and merges:
    val = max(S'_1 - 32, relu(S'_2 - 32)).
  - two pairwise fp16 max folds shrink the row 2048 -> 512 before the DVE
    `max` (hardware top-8, descending) extracts the first <=8 matching j;
    max never alters values, so survivors still encode j exactly. A fold
    loses a match only if two nonzeros share a fold group, which is
    detected exactly by sum conservation (accum_out sums are fp32-exact;
    sum(val) > sum(M2) iff some fold had two positives) and turns into a
    forced-positive 8th slot.
  - three 2-source ops decode the top-8 values to j / -1.
  - rows whose 8th candidate decodes as a real match (>=8 real matches, or
    the collision flag) are recomputed exactly on the host with numpy. With
    random normal inputs this never triggers: a match needs a 2^-32
    sign-pattern collision.

Self-contained: hardcodes shapes from the problem spec.
"""

import numpy as np

B = 4
L = 2048
D = 64
K_MAX = 64
N_CORES = 8
QSH = B * L // N_CORES  # 1024 queries per core
N_QT = QSH // 128       # 8 query tiles per core

_CACHE = {}


def _build_program(reps=1):
    from contextlib import ExitStack

    import concourse.bacc as bacc
    import concourse.mybir as mybir
    import concourse.tile as tile

    dt = mybir.dt
    Alu = mybir.AluOpType

    # Bacc (not raw Bass): its legalization passes split multi-sem waits,
    # which PE instructions can't carry (1 wait max per instruction).
    nc = bacc.Bacc("TRN2", target_bir_lowering=False, debug=False)
    qT_d = nc.declare_dram_parameter("qT", [D, QSH], dt.float32, isOutput=False)
    kT_d = nc.declare_dram_parameter("kT", [D, L], dt.float32, isOutput=False)
    ramp_d = nc.declare_dram_parameter("ramp", [2, L], dt.bfloat16, isOutput=False)
    out_d = nc.declare_dram_parameter("out", [QSH, K_MAX], dt.int32, isOutput=True)

    with tile.TileContext(nc) as tc, ExitStack() as ctx:
        consts = ctx.enter_context(tc.tile_pool(name="consts", bufs=1))
        vals = ctx.enter_context(tc.tile_pool(name="vals", bufs=2))
        outs = ctx.enter_context(tc.tile_pool(name="outs", bufs=1))
        psum = ctx.enter_context(tc.tile_pool(name="psum", bufs=2, space="PSUM"))

        # ---- load raw (transposed) inputs ----
        qraw = consts.tile([D, QSH], dt.float32)
        kraw = consts.tile([D, L], dt.float32)
        nc.sync.dma_start(qraw[:], qT_d[:])
        nc.sync.dma_start(kraw[:], kT_d[:])

        # per-partition bias constant for the relu evacuation
        bias32 = consts.tile([128, 1], dt.float32, tag="bias32")
        nc.vector.memset(bias32[:], -32.0)
        # decode constants (tiles so decode ops can be 2-source 1x-mode ops)
        c2048 = consts.tile([128, 64], dt.float32, tag="c2048")
        nc.vector.memset(c2048[:], 2048.0)
        z64 = consts.tile([128, 64], dt.float32, tag="z64")
        nc.vector.memset(z64[:], 0.0)
        # -1 padding for output columns 8..63
        pad56 = consts.tile([128, K_MAX - 8], dt.int32, tag="pad56")
        nc.vector.memset(pad56[:], -1)
        # all 8 query tiles' top-8 values, decoded in one shot at the end
        t8all = consts.tile([128, 64], dt.float16, tag="t8all")

        # ---- sign tiles (+ ramp rows) ----
        # QS[g]: [34, QSH]  rows 0:32 = signs of dims g*32:(g+1)*32,
        #                   rows 32/33 = 1.0 (ramp passthrough weights)
        # KS[g]: [34, L]    rows 0:32 = key signs, rows 32/33 = ramp terms
        QS = []
        KS = []
        # sign bias: sign(x - 1e-20) == 2*(x>0)-1 for every fp32 value the
        # randn inputs can take (smallest nonzero magnitude ~3e-7), and maps
        # x == 0.0 to -1 exactly like the reference's (x > 0).
        eps_b = consts.tile([64, 1], dt.float32, tag="eps_b")
        nc.vector.memset(eps_b[:], -1e-20)

        def sign_dve(dst, src):
            # s = ((x > 0)*2) - 1 in two DVE tensor_scalar passes (exact,
            # including x == 0 -> -1, matching the reference's (x > 0))
            nc.vector.tensor_scalar(
                out=dst, in0=src,
                scalar1=0.0, scalar2=2.0, op0=Alu.is_gt, op1=Alu.mult)
            nc.vector.tensor_scalar(
                out=dst, in0=dst, scalar1=-1.0, scalar2=None, op0=Alu.add)

        for g in range(2):
            qs = consts.tile([34, QSH], dt.bfloat16, tag=f"qs{g}")
            ks = consts.tile([34, L], dt.bfloat16, tag=f"ks{g}")
            lo, hi = g * 32, (g + 1) * 32
            # Sign prep gates the first matmuls: split it DVE (3 tensors,
            # exact is_gt path) / ACT (keys group 2, Sign(x - eps) which
            # equals 2*(x>0)-1 for every reachable fp32 randn value).
            sign_dve(qs[0:32, :], qraw[lo:hi, :])
            if g == 0:
                sign_dve(ks[0:32, :], kraw[lo:hi, :])
            else:
                nc.scalar.activation(
                    ks[0:32, :], kraw[lo:hi, :],
                    mybir.ActivationFunctionType.Sign,
                    bias=eps_b[0:32, :], scale=1.0)
            nc.vector.memset(qs[32:34, :], 1.0)
            # ramp terms (host-precomputed bf16 constants) into rows 32/33
            nc.sync.dma_start(ks[32:34, :], ramp_d[:])
            QS.append(qs)
            KS.append(ks)

        # ---- main loop over query tiles ----
        # reps>1 repeats the whole body inside one NEFF (timing only).
        for t in [qt for _ in range(reps) for qt in range(N_QT)]:
            # ScalarE evacuates group 2 with relu(S'_2 - 32) -> fp16 (matched
            # positions give exactly (2048-j)*2^-13, everything else 0);
            # DVE then fuses group 1's evacuation with the merge:
            # val = (S'_1 - 32) max relu(S'_2 - 32) == relu(max(S'_1,S'_2)-32)
            # for the matched range, since all matched values are > 0.
            v2 = vals.tile([128, L], dt.float16, tag="v2")
            val = vals.tile([128, L], dt.float16, tag="val")
            sv = vals.tile([128, 2], dt.float32, tag="sv")
            for h in range(2):  # halves of the key axis
                p0 = psum.tile([128, 1024], dt.float32, tag="p0")
                p1 = psum.tile([128, 1024], dt.float32, tag="p1")
                for g, pg in enumerate((p0, p1)):
                    for n in range(2):
                        nc.tensor.matmul(
                            pg[:, n * 512:(n + 1) * 512],
                            QS[g][:, t * 128:(t + 1) * 128],
                            KS[g][:, h * 1024 + n * 512: h * 1024 + (n + 1) * 512],
                            start=True, stop=True)
                cols = slice(h * 1024, (h + 1) * 1024)
                nc.scalar.activation(
                    v2[:, cols], p1[:], mybir.ActivationFunctionType.Relu,
                    bias=bias32[:], scale=1.0)
                # accum_out gives sum(val half) for free (exact in fp32:
                # all values are multiples of 2^-13 bounded by 512)
                nc.vector.scalar_tensor_tensor(
                    out=val[:, cols], in0=p0[:], scalar=-32.0,
                    in1=v2[:, cols], op0=Alu.add, op1=Alu.max,
                    accum_out=sv[:, h:h + 1])

            # Shrink the top-8 scan 2048 -> 512 with two pairwise max folds.
            # A fold only loses information if both elements of a pair are
            # nonzero ("collision"); then sum(M2) < sum(val) strictly, which
            # the fp32-exact sums detect. max() never alters values, so the
            # surviving entries still encode j exactly.
            m1 = vals.tile([128, L // 2], dt.float16, tag="m1")
            nc.vector.tensor_tensor(
                out=m1[:], in0=val[:, 0:1024], in1=val[:, 1024:2048],
                op=Alu.max)
            m2 = vals.tile([128, L // 4], dt.float16, tag="m2")
            sm = vals.tile([128, 1], dt.float32, tag="sm")
            nc.vector.scalar_tensor_tensor(
                out=m2[:], in0=m1[:, 0:512], scalar=0.0,
                in1=m1[:, 512:1024], op0=Alu.add, op1=Alu.max,
                accum_out=sm[:])

            # top-8 values per query row, descending == first <=8 matches
            nc.vector.max(t8all[:, 8 * t:8 * t + 8], m2[:])

            # collision flag -> force slot 7 positive, which triggers the
            # same exact host fallback as the >8-matches case.
            svt = vals.tile([128, 1], dt.float32, tag="svt")
            nc.vector.tensor_tensor(
                out=svt[:], in0=sv[:, 0:1], in1=sv[:, 1:2], op=Alu.add)
            flag = vals.tile([128, 1], dt.float32, tag="flag")
            nc.vector.tensor_tensor(
                out=flag[:], in0=svt[:], in1=sm[:], op=Alu.is_gt)
            nc.vector.scalar_tensor_tensor(
                out=t8all[:, 8 * t + 7:8 * t + 8], in0=flag[:],
                scalar=2.0 ** -13, in1=t8all[:, 8 * t + 7:8 * t + 8],
                op0=Alu.mult, op1=Alu.max)

        # ---- decode all tiles at once ----
        # matched v = (2048-j)*2^-13 => u = 2048 - 8192*v = j in [0, 2047];
        # unmatched v = 0 => u = 2048 -> -1.
        u = outs.tile([128, 64], dt.float32, tag="u")
        nc.vector.scalar_tensor_tensor(
            out=u[:], in0=t8all[:], scalar=-8192.0, in1=c2048[:],
            op0=Alu.mult, op1=Alu.add)
        pad = outs.tile([128, 64], dt.float32, tag="pad")
        # pad = relu(u - 2047): 1 iff u == 2048 (unmatched), else 0
        nc.vector.scalar_tensor_tensor(
            out=pad[:], in0=u[:], scalar=-2047.0, in1=z64[:],
            op0=Alu.add, op1=Alu.max)
        # o = u - 2049*pad  -> j or -1 (int32 cast on write)
        o = outs.tile([128, 64], dt.int32, tag="o")
        nc.vector.scalar_tensor_tensor(
            out=o[:], in0=pad[:], scalar=-2049.0, in1=u[:],
            op0=Alu.mult, op1=Alu.add)
        for t in range(N_QT):
            nc.sync.dma_start(out_d[t * 128:(t + 1) * 128, 0:8],
                              o[:, 8 * t:8 * t + 8])
            nc.sync.dma_start(out_d[t * 128:(t + 1) * 128, 8:K_MAX], pad56[:])

    return nc


def _get_program():
    if "prog" not in _CACHE:
        nc = _build_program()
        if not nc.is_finalized():
            nc.finalize()  # Bacc: runs wait-splitting + reg-alloc passes
        _CACHE["prog"] = nc
    return _CACHE["prog"]


def _ramp_rows():
    """[2, L] bf16 rows summing (via the all-ones weight rows) to
    ramp(j) = (2048-j)*2^-13: hi = (128-(j>>4))*2^-9, lo = -(j&15)*2^-13.
    Every term is exactly representable in bf16, and relu(S'-32) lands in
    (0, 0.25] where fp16 spacing is <= 2^-13, so values stay exact."""
    import ml_dtypes
    j = np.arange(L)
    hi = (128 - (j >> 4)).astype(np.float32) * 2.0 ** -9
    lo = -(j & 15).astype(np.float32) * 2.0 ** -13
    return np.stack([hi, lo]).astype(ml_dtypes.bfloat16)


def _make_in_maps(q, k):
    ramp = _ramp_rows()
    in_maps = []
    for c in range(N_CORES):
        b, h = divmod(c, 2)
        qT = np.ascontiguousarray(q[b, h * QSH:(h + 1) * QSH, :].T)
        kT = np.ascontiguousarray(k[b].T)
        in_maps.append({"qT": qT, "kT": kT, "ramp": ramp})
    return in_maps


def run_device(q, k, trace=False):
    """Run the bass kernel on the 8 cores; returns (full_out, results_obj)."""
    from concourse.bass_utils import run_bass_kernel_spmd

    res = run_bass_kernel_spmd(
        _get_program(), _make_in_maps(q, k), list(range(N_CORES)), trace=trace)
    full = np.empty((B, L, K_MAX), np.int32)
    for c in range(N_CORES):
        b, h = divmod(c, 2)
        full[b, h * QSH:(h + 1) * QSH, :] = res.results[c]["out"]
    return full, res


def _reference_numpy(q, k):
    """Exact numpy fallback (used only if some row has >= 8 matches)."""
    out = np.full((B, L, K_MAX), -1, np.int32)
    for b in range(B):
        qb = (q[b] > 0)
        kb = (k[b] > 0)
        match = np.zeros((L, L), bool)
        for lo in (0, 32):
            qg = qb[:, lo:lo + 32]
            kg = kb[:, lo:lo + 32]
            # pack 32 bits into one uint32 per row for exact equality
            qc = np.packbits(qg, axis=1).view(">u4").ravel()
            kc = np.packbits(kg, axis=1).view(">u4").ravel()
            match |= qc[:, None] == kc[None, :]
        for i in range(L):
            idx = np.nonzero(match[i])[0][:K_MAX]
            out[b, i, :len(idx)] = idx
    return out


def kernel(query_up, key_up, head_idx=None, **_unused):
    q = np.asarray(query_up, dtype=np.float32)
    k = np.asarray(key_up, dtype=np.float32)
    assert q.shape == (B, L, D) and k.shape == (B, L, D)
    full, _ = run_device(q, k)
    # Exact overflow detection: a non(-1) 8th candidate means the row had
    # >= 8 matches, so candidates 9.. might have been dropped.
    if (full[..., 7] != -1).any():
        full = _reference_numpy(q, k)
    return full
